# revision 13
# baseline (speedup 1.0000x reference)
"""Multi-head attention (B=4, S=2048, D=1024, H=16) on 8 trn2 NeuronCores.

Sharding: core c -> batch b = c//2, head-group hg = c%2 (8 heads, 512 feature
dims per core).  Each core computes its batch's attention for its 8 heads plus
the partial output projection; the host sums the two partials per batch and
adds the output bias.

Device-side layout (per core):
  inputs transposed on host: X^T [D, S] for query/key/value of its batch.
  Q^T, K^T d-major [512, S] as 4 head-pair tiles [128, 2048]  (f32r)
  V token-major [S, 520]: per head 64 value cols + 1 "ones" col (from an
     augmented Wv with a zero column and bias 1) -> gives softmax denominators
     for free during the context matmul (M=65).
  scores^T [k, q] via packed K=64 matmul pairs (two heads concurrently in the
     PE array via tile_position row groups).
  exp fused on ACT: exp(score*0.125 + mask_bias[k]) with the additive mask as
     a per-partition activation bias.
  context^T accumulated over 16 k-chunks in PSUM [65, 512]; row 64 = denom.
  normalize: reciprocal -> DRAM bounce -> partition-broadcast -> DVE multiply.
  output projection from head-pair-stacked normalized context chunks (K=128).
"""

import numpy as np

B, S, D = 4, 2048, 1024
H, DK = 16, 64
NCORES = 8
DS = 512          # feature dims per core (8 heads)
FCH = 8           # feature chunks of 128 in D
DT = 4            # d-tiles (head pairs) per core
QB = 4            # q blocks of 512
KT = 16           # k tiles of 128
TT = 16           # token tiles of 128

_cache = {}


def _build_nc(niter=1):
    import concourse.bass as bass  # noqa: F401
    import concourse.mybir as mybir
    from concourse import bacc
    from concourse.tile import TileContext
    from contextlib import nullcontext

    f32 = mybir.dt.float32
    f32r = mybir.dt.float32r
    EXP = mybir.ActivationFunctionType.Exp

    nc = bacc.Bacc(None, target_bir_lowering=False)
    qt_in = nc.declare_dram_parameter("qt", [D, S], f32, isOutput=False)
    kt_in = nc.declare_dram_parameter("kt", [D, S], f32, isOutput=False)
    vt_in = nc.declare_dram_parameter("vt", [D, S], f32, isOutput=False)
    wq_in = nc.declare_dram_parameter("wq", [D, DS], f32, isOutput=False)
    wk_in = nc.declare_dram_parameter("wk", [D, DS], f32, isOutput=False)
    wv_in = nc.declare_dram_parameter("wv", [D, 520], f32, isOutput=False)
    wo_in = nc.declare_dram_parameter("wo", [DS, D], f32, isOutput=False)
    bq_in = nc.declare_dram_parameter("bq", [128, DT], f32, isOutput=False)
    bk_in = nc.declare_dram_parameter("bk", [128, DT], f32, isOutput=False)
    bvr_in = nc.declare_dram_parameter("bvr", [128, 520], f32, isOutput=False)
    mb_in = nc.declare_dram_parameter("mb", [128, KT], f32, isOutput=False)
    out_d = nc.declare_dram_parameter("out", [S, D], f32, isOutput=True)
    rscr = nc.dram_tensor("rscr", [H // 2 * QB * 2, 512], f32)

    with TileContext(nc) as tc:
        with (
            tc.For_i(0, niter, 1) if niter > 1 else nullcontext(),
            tc.tile_pool(name="keep", bufs=1) as keep,
            tc.tile_pool(name="sc", bufs=2, space="PSUM") as pssc,
            tc.tile_pool(name="acc", bufs=4, space="PSUM") as psacc,
        ):
            # ---- small constants ----
            bq_sb = keep.tile([128, DT], f32)
            bk_sb = keep.tile([128, DT], f32)
            bvr_sb = keep.tile([128, 520], f32)
            mb_sb = keep.tile([128, KT], f32)
            nc.sync.dma_start(out=bq_sb, in_=bq_in[:, :])
            nc.sync.dma_start(out=bk_sb, in_=bk_in[:, :])
            nc.sync.dma_start(out=bvr_sb, in_=bvr_in[:, :])
            nc.sync.dma_start(out=mb_sb, in_=mb_in[:, :])

            qt_sb = [keep.tile([128, S], f32r, tag="qt", bufs=DT, name=f"qt{t}") for t in range(DT)]
            kt_sb = [keep.tile([128, S], f32r, tag="kt", bufs=DT, name=f"kt{t}") for t in range(DT)]
            v_sb = [keep.tile([128, 520], f32r, tag="v", bufs=TT, name=f"v{t}") for t in range(TT)]

            with tc.tile_pool(name="proj", bufs=1) as proj:
                # ---- V projection FIRST (attention's context matmuls need all
                # of V; finishing it early unblocks the attention pipeline) ----
                wv_sb = proj.tile([128, FCH, 520], f32r, tag="w", bufs=2)
                nc.sync.dma_start(
                    out=wv_sb,
                    in_=wv_in.ap().rearrange("(c p) d -> p c d", p=128).bitcast(f32r),
                )
                vc = []
                for c in range(FCH):
                    v_t = proj.tile([128, S], f32r, tag="xt", bufs=FCH, name=f"xv{c}")
                    nc.sync.dma_start(
                        out=v_t, in_=vt_in[c * 128:(c + 1) * 128, :].bitcast(f32r)
                    )
                    vc.append(v_t)
                for tt in range(TT):
                    vps = pssc.tile([128, 520], f32, tag="sc", name=f"vps{tt}")
                    for c in range(FCH):
                        nc.tensor.matmul(
                            vps[:, 0:512], vc[c][:, tt * 128:(tt + 1) * 128],
                            wv_sb[:, c, 0:512],
                            start=(c == 0), stop=(c == FCH - 1),
                        )
                        nc.tensor.matmul(
                            vps[:, 512:520], vc[c][:, tt * 128:(tt + 1) * 128],
                            wv_sb[:, c, 512:520],
                            start=(c == 0), stop=(c == FCH - 1),
                        )
                    nc.vector.tensor_add(v_sb[tt], vps, bvr_sb)

                # ---- Q^T then K^T projections (d-major), t-outer so each
                # head-pair's tiles complete early and attention can overlap
                # the projection tail.  K uses only 2 PSUM accumulators to
                # leave slots for the context accumulators during overlap. ----
                for name, w_dram, x_dram, b_sb, o_tiles, nacc in (
                    ("q", wq_in, qt_in, bq_sb, qt_sb, 4),
                    ("k", wk_in, kt_in, bk_sb, kt_sb, 2),
                ):
                    w_sb = proj.tile([128, FCH, DS], f32r, tag="w", bufs=2, name=f"w{name}")
                    nc.sync.dma_start(
                        out=w_sb,
                        in_=w_dram.ap().rearrange("(c p) d -> p c d", p=128).bitcast(f32r),
                    )
                    xc = []
                    for c in range(FCH):
                        x_t = proj.tile([128, S], f32r, tag="xt", bufs=FCH, name=f"x{name}{c}")
                        nc.sync.dma_start(
                            out=x_t, in_=x_dram[c * 128:(c + 1) * 128, :].bitcast(f32r)
                        )
                        xc.append(x_t)
                    for t in range(DT):
                        for qg in range(QB // nacc):
                            qbs = range(qg * nacc, (qg + 1) * nacc)
                            accs = {
                                qb: psacc.tile([128, 512], f32, tag="acc",
                                               name=f"pa{name}{t}{qb}")
                                for qb in qbs
                            }
                            for c in range(FCH):
                                for qb in qbs:
                                    nc.tensor.matmul(
                                        accs[qb],
                                        w_sb[:, c, t * 128:(t + 1) * 128],
                                        xc[c][:, qb * 512:(qb + 1) * 512],
                                        start=(c == 0),
                                        stop=(c == FCH - 1),
                                    )
                            for qb in qbs:
                                nc.vector.tensor_scalar_add(
                                    o_tiles[t][:, qb * 512:(qb + 1) * 512],
                                    accs[qb],
                                    b_sb[:, t:t + 1],
                                )

            # ---- attention + context, per head-pair and q-block ----
            with tc.tile_pool(name="attn", bufs=1) as attn:
                wo_sb = attn.tile([128, DT, D], f32r, tag="wo", bufs=1)
                nc.sync.dma_start(
                    out=wo_sb,
                    in_=wo_in.ap().rearrange("(h p) n -> p h n", p=128).bitcast(f32r),
                )
                cn_sb = [attn.tile([128, S], f32r, tag="cn", bufs=DT, name=f"cn{h}") for h in range(DT)]

                for hp in range(DT):
                    for qb in range(QB):
                        acc0 = psacc.tile([65, 512], f32, tag="acc", name=f"ca0_{hp}{qb}")
                        acc1 = psacc.tile([65, 512], f32, tag="acc", name=f"ca1_{hp}{qb}")
                        for k in range(KT):
                            sct = pssc.tile([128, 1024], f32, tag="sc", name=f"sct{hp}{qb}{k}")
                            nc.tensor.matmul(
                                sct[:, 0:512],
                                kt_sb[hp][0:64, k * 128:(k + 1) * 128],
                                qt_sb[hp][0:64, qb * 512:(qb + 1) * 512],
                                start=True, stop=True, tile_position=(0, 0),
                            )
                            nc.tensor.matmul(
                                sct[:, 512:1024],
                                kt_sb[hp][64:128, k * 128:(k + 1) * 128],
                                qt_sb[hp][64:128, qb * 512:(qb + 1) * 512],
                                start=True, stop=True, tile_position=(64, 0),
                            )
                            et = attn.tile([128, 1024], f32r, tag="et", bufs=4, name=f"et{hp}{qb}{k}")
                            nc.scalar.activation(
                                out=et, in_=sct, func=EXP,
                                bias=mb_sb[:, k:k + 1], scale=0.125,
                            )
                            lh0, lh1 = 2 * hp, 2 * hp + 1
                            nc.tensor.matmul(
                                acc0, v_sb[k][:, lh0 * 65:lh0 * 65 + 65], et[:, 0:512],
                                start=(k == 0), stop=(k == KT - 1),
                            )
                            nc.tensor.matmul(
                                acc1, v_sb[k][:, lh1 * 65:lh1 * 65 + 65], et[:, 512:1024],
                                start=(k == 0), stop=(k == KT - 1),
                            )
                        for half, acc in ((0, acc0), (1, acc1)):
                            idx = (hp * QB + qb) * 2 + half
                            r1 = attn.tile([128, 512], f32, tag="r1", bufs=4, name=f"r1_{idx}")
                            nc.vector.reciprocal(r1[64:65, :], acc[64:65, :])
                            nc.sync.dma_start(out=rscr[idx, :], in_=r1[64:65, :])
                            rr = attn.tile([64, 512], f32, tag="rr", bufs=4, name=f"rr_{idx}")
                            nc.sync.dma_start(
                                out=rr, in_=rscr[idx, :].unsqueeze(0).partition_broadcast(64)
                            )
                            if half == 0:
                                nc.vector.tensor_mul(
                                    cn_sb[hp][0:64, qb * 512:(qb + 1) * 512], acc[0:64, :], rr
                                )
                            else:
                                tm = attn.tile([64, 512], f32r, tag="tm", bufs=4, name=f"tm_{idx}")
                                nc.vector.tensor_mul(tm, acc[0:64, :], rr)
                                nc.sync.dma_start(
                                    out=cn_sb[hp][64:128, qb * 512:(qb + 1) * 512], in_=tm
                                )

                # ---- output projection (lhsT reused across both n-blocks) ----
                for qt_i in range(TT):
                    pos = [
                        psacc.tile([128, 512], f32, tag="acc", name=f"po{qt_i}{nb}")
                        for nb in range(2)
                    ]
                    for hp in range(DT):
                        for nb in range(2):
                            nc.tensor.matmul(
                                pos[nb],
                                cn_sb[hp][:, qt_i * 128:(qt_i + 1) * 128],
                                wo_sb[:, hp, nb * 512:(nb + 1) * 512],
                                start=(hp == 0), stop=(hp == DT - 1),
                            )
                    for nb in range(2):
                        os_t = attn.tile([128, 512], f32, tag="os", bufs=4, name=f"os{qt_i}{nb}")
                        nc.vector.tensor_copy(os_t, pos[nb])
                        nc.sync.dma_start(
                            out=out_d[qt_i * 128:(qt_i + 1) * 128, nb * 512:(nb + 1) * 512],
                            in_=os_t,
                        )

    nc.finalize()
    return nc


def _get_nc(niter=1):
    key = ("nc", niter)
    if key not in _cache:
        _cache[key] = _build_nc(niter)
    return _cache[key]


def _make_in_maps(query, key, value, mask, Wq, bq, Wk, bk, Wv, bv, Wo, bo):
    f = np.float32
    in_maps = []
    for c in range(NCORES):
        b, hg = c // 2, c % 2
        hs = hg * DS
        wv_aug = np.zeros((D, 520), f)
        bvr_row = np.zeros((520,), f)
        for lh in range(8):
            wv_aug[:, lh * 65:lh * 65 + 64] = Wv[:, hs + lh * 64: hs + (lh + 1) * 64]
            bvr_row[lh * 65:lh * 65 + 64] = bv[hs + lh * 64: hs + (lh + 1) * 64]
            bvr_row[lh * 65 + 64] = 1.0
        mbias = np.where(mask[b, 0, 0, :] == 0, f(-1e9), f(0.0)).astype(f)
        in_maps.append({
            "qt": np.ascontiguousarray(query[b].T, dtype=f),
            "kt": np.ascontiguousarray(key[b].T, dtype=f),
            "vt": np.ascontiguousarray(value[b].T, dtype=f),
            "wq": np.ascontiguousarray(Wq[:, hs:hs + DS], dtype=f),
            "wk": np.ascontiguousarray(Wk[:, hs:hs + DS], dtype=f),
            "wv": wv_aug,
            "wo": np.ascontiguousarray(Wo[hs:hs + DS, :], dtype=f),
            "bq": np.ascontiguousarray(bq[hs:hs + DS].reshape(DT, 128).T, dtype=f),
            "bk": np.ascontiguousarray(bk[hs:hs + DS].reshape(DT, 128).T, dtype=f),
            "bvr": np.tile(bvr_row[None, :], (128, 1)).astype(f),
            "mb": np.ascontiguousarray(mbias.reshape(KT, 128).T, dtype=f),
        })
    return in_maps


def kernel(query, key, value, mask, Wq, bq, Wk, bk, Wv, bv, Wo, bo):
    from concourse.bass_utils import run_bass_kernel_spmd

    args = [np.asarray(a) for a in (query, key, value, mask, Wq, bq, Wk, bk, Wv, bv, Wo, bo)]
    query, key, value, mask, Wq, bq, Wk, bk, Wv, bv, Wo, bo = args
    nc = _get_nc()
    in_maps = _make_in_maps(query, key, value, mask, Wq, bq, Wk, bk, Wv, bv, Wo, bo)
    res = run_bass_kernel_spmd(nc, in_maps, list(range(NCORES)))
    out = np.empty((B, S, D), np.float32)
    for b in range(B):
        out[b] = res.results[2 * b]["out"] + res.results[2 * b + 1]["out"] + bo[None, :]
    return out
